# revision 21
# baseline (speedup 1.0000x reference)
"""Trainium2 Bass kernel for nn_MeshConv (COO SpMM + 128x128 Linear).

out[r, :] = (sum_{e: rows[e]==r} vals[e] * x[cols[e], :]) @ W.T + b

Strategy (8 NeuronCores, one SPMD program):
  - Row-shard: core c owns output rows [c*12500, (c+1)*12500); x, W, b
    are replicated per core, so no collectives are needed.
  - Host packs each core's edges densely per (batch of windows, 32768-row
    column chunk) into 128-slot gather tiles, sorted by output window.
    Per-core shortfall vs the SPMD max is padded at the call tail with
    idx=-1 (the gather ucode skips trailing negatives).
  - The selection matrices S[slot, row] = val_e * onehot(lrow_e) are
    fully PRECOMPUTED ON THE HOST (indices and values are kernel inputs,
    so S is just data) and streamed from HBM — no on-device S build.
  - Device, per batch: dma_gather x rows per chunk into SBUF (bf16),
    stream S tiles, and accumulate aggT[cin, row] = sum_t Xg_t^T @ S_t
    per window in PSUM on TensorE.  Then outT_w = W @ agg = wt^T @ aggT
    with one more matmul, and the Scalar engine fuses the per-partition
    bias while copying PSUM -> SBUF.  Output is stored transposed
    [C, rows] and untransposed on the host.
"""

import os
import sys

for _p in ("/opt/trn_rl_repo",):
    if _p not in sys.path:
        sys.path.insert(0, _p)

import numpy as np

# --- problem constants ---
N_NODES = 100000
C = 128
N_CORES = 8
RPC = N_NODES // N_CORES          # rows per core: 12500
WIN = 128                         # output window = PSUM partition dim
NW = (RPC + WIN - 1) // WIN       # 98 windows (last has 84 rows)
CHUNK = 32768                     # column chunk (int16 gather indices)
NK = (N_NODES + CHUNK - 1) // CHUNK  # 4

CB = int(os.environ.get("MESHCONV_CB", "160"))        # gather tile cols per batch
OCCCAP = int(os.environ.get("MESHCONV_OCCCAP", "400"))  # S cols per batch (loose)
NSWQ = int(os.environ.get("MESHCONV_NSWQ", "4"))
SINGLE_PACKET = os.environ.get("MESHCONV_SP", "0") == "1"

TRACE = False          # set by test.py for profiling runs
LAST_RESULT = {}       # test.py reads exec_time_ns etc. from here


def _host_prep(rows, cols, vals):
    """Pack edges densely per (core, batch, chunk); precompute S tiles.

    Returns:
      eidx16  [NC, 128, TC*8] int16   wrapped gather-index planes
      s_all   [NC, 128, OCC*WIN] bf16-able f32  selection matrices
      batches list of (w0, nwin, tile_cols, occ0, nocc)
      calls   list per batch of (k, tile_col_base, ntiles)
      sched   per window: list of (xg_col, occ_idx) matmul operands
      tc_total, occ_total
    """
    rows = np.asarray(rows).astype(np.int64)
    cols = np.asarray(cols).astype(np.int64)
    vals = np.asarray(vals).astype(np.float32)
    E = len(rows)

    core = rows // RPC
    lr = rows - core * RPC
    win = lr // WIN
    lrow = lr - win * WIN
    chunk = cols // CHUNK
    cidx = cols - chunk * CHUNK

    # counts per (core, window, chunk)
    gid = (core * NW + win) * NK + chunk
    cnt = np.bincount(gid, minlength=N_CORES * NW * NK).reshape(N_CORES, NW, NK)

    # --- greedy batching of consecutive windows ---
    # For a candidate batch [w0, w): per chunk k, per core c the edges are
    # packed densely; tiles(k) = ceil(max_c sum_w cnt/128).  Occurrences:
    # for each (w, k) the union tile range over cores.
    def batch_stats(w0, w1):
        tiles = np.zeros(NK, dtype=np.int64)
        nocc = 0
        for k in range(NK):
            tot = cnt[:, w0:w1, k].sum(axis=1)           # [NC]
            tiles[k] = -(-tot.max() // 128)
            cum = np.cumsum(cnt[:, w0:w1, k], axis=1)    # inclusive
            start = cum - cnt[:, w0:w1, k]
            end = cum
            any_e = (cnt[:, w0:w1, k].max(axis=0) > 0)
            lo = np.where(any_e, (start // 128).min(axis=0), 0)
            hi = np.where(any_e, -(-end.max(axis=0) // 128), 0)
            nocc += int(np.maximum(hi - lo, 0).sum())
        return int(tiles.sum()), nocc, tiles

    batches_w = []
    w = 0
    while w < NW:
        w0 = w
        w1 = w0 + 1
        while w1 < NW:
            tc, no, _ = batch_stats(w0, w1 + 1)
            if tc > CB or no > OCCCAP:
                break
            w1 += 1
        tc, no, tiles = batch_stats(w0, w1)
        assert tc <= CB and no <= OCCCAP, (w0, w1, tc, no)
        batches_w.append((w0, w1, tiles))
        w = w1


    # --- global tile-column / occurrence layout ---
    batches = []
    calls = []          # per batch: list of (k, col_base, ntiles)
    sched = [None] * NW  # per window: list of (xg_local_col, occ_local, global refs)
    tilecolbase = np.full((len(batches_w), NK), -1, dtype=np.int64)
    occ_base = {}       # (b, w, k) -> (occ_global_base, lo_tile)
    tc_total = 0
    occ_total = 0
    for bi, (w0, w1, tiles) in enumerate(batches_w):
        bcalls = []
        bt0 = tc_total
        bo0 = occ_total
        for k in range(NK):
            if tiles[k] > 0:
                tilecolbase[bi, k] = tc_total
                bcalls.append((k, tc_total, int(tiles[k])))
                tc_total += int(tiles[k])
        # occurrences ordered by (w, k, t)
        for w_ in range(w0, w1):
            for k in range(NK):
                if tiles[k] == 0:
                    continue
                tot = cnt[:, w0:w1, k]
                cum = np.cumsum(tot, axis=1)
                start = cum[:, w_ - w0] - tot[:, w_ - w0]
                end = cum[:, w_ - w0]
                if tot[:, w_ - w0].max() == 0:
                    continue
                lo = int((start // 128).min())
                hi = int(-(-end.max() // 128))
                occ_base[(bi, w_, k)] = (occ_total, lo)
                occ_total += hi - lo
        batches.append((w0, w1 - w0, tc_total - bt0, bo0, occ_total - bo0))
        calls.append(bcalls)

    # --- slot assignment for every edge ---
    wb = np.zeros(NW, dtype=np.int64)           # window -> batch
    for bi, (w0, w1, _) in enumerate(batches_w):
        wb[w0:w1] = bi
    eb = wb[win]
    # sort edges by (core, batch, chunk, window); rank within call = slot
    order = np.lexsort((win, chunk, eb, core))
    core_s = core[order]
    eb_s = eb[order]
    chunk_s = chunk[order]
    callid = (core_s * len(batches) + eb_s) * NK + chunk_s
    ncalls = N_CORES * len(batches) * NK
    start_of = np.searchsorted(callid, np.arange(ncalls), side="left")
    slot = np.arange(E) - start_of[callid]

    # per-call max real count -> per-core pad at the tail
    call_cnt = np.bincount(callid, minlength=ncalls).reshape(
        N_CORES, len(batches), NK
    )

    tile_s = slot // 128
    p_s = slot - tile_s * 128

    # vectorized lookup tables [NW, NK]
    tcb_wk = np.zeros((NW, NK), dtype=np.int64)       # tile col base
    occ_base_wk = np.full((NW, NK), -1, dtype=np.int64)
    occ_lo_wk = np.zeros((NW, NK), dtype=np.int64)
    occ_hi_wk = np.zeros((NW, NK), dtype=np.int64)
    for bi, (w0, w1, tiles) in enumerate(batches_w):
        for k in range(NK):
            tcb_wk[w0:w1, k] = tilecolbase[bi, k]
    for (bi, w_, k), (ob, lo) in occ_base.items():
        occ_base_wk[w_, k] = ob
        occ_lo_wk[w_, k] = lo
    for bi, (w0, w1, tiles) in enumerate(batches_w):
        for k in range(NK):
            tot = cnt[:, w0:w1, k]
            cum = np.cumsum(tot, axis=1)
            start = cum - tot
            end = cum
            any_e = tot.max(axis=0) > 0
            hi = np.where(any_e, -(-end.max(axis=0) // 128), 0)
            occ_hi_wk[w0:w1, k] = hi

    # gather index planes: [NC, tc_total, 128] then wrapped.
    # Padding uses idx=0 (gathers row kb+0, masked by S=0): trailing -1
    # truncation would desync the decode-side ring bookkeeping (which uses
    # num_idxs_reg) from the ucode's actual descriptor count.
    sidx = np.zeros((N_CORES, tc_total, 128), dtype=np.int16)
    win_s = win[order]
    gcol = tcb_wk[win_s, chunk_s] + tile_s
    sidx[core_s, gcol, p_s] = cidx[order].astype(np.int16)

    eidx16 = np.zeros((N_CORES, 128, tc_total * 8), dtype=np.int16)
    flat = sidx.reshape(N_CORES, tc_total * 128)
    wrapped = flat.reshape(N_CORES, tc_total * 8, 16).transpose(0, 2, 1)
    eidx16[:] = np.tile(wrapped, (1, 8, 1))

    # --- S matrices ---
    occ_e = occ_base_wk[win_s, chunk_s] + tile_s - occ_lo_wk[win_s, chunk_s]
    assert (occ_base_wk[win_s, chunk_s] >= 0).all()
    assert (occ_e >= 0).all() and (occ_e < occ_total).all()
    assert (tile_s >= occ_lo_wk[win_s, chunk_s]).all()
    assert (tile_s < occ_hi_wk[win_s, chunk_s]).all()
    import ml_dtypes

    lrow_s = lrow[order]
    vals_s = vals[order]
    s_all = []
    for c in range(N_CORES):
        m = core_s == c
        sc = np.zeros((occ_total, 128, WIN), dtype=np.float32)
        sc[occ_e[m], p_s[m], lrow_s[m]] = vals_s[m]
        s_all.append(
            np.ascontiguousarray(
                sc.transpose(1, 0, 2).reshape(128, occ_total * WIN)
            ).astype(ml_dtypes.bfloat16)
        )

    # --- matmul schedule per window ---
    for w_ in range(NW):
        ops = []
        for k in range(NK):
            if occ_base_wk[w_, k] < 0:
                continue
            ob = occ_base_wk[w_, k]
            lo = occ_lo_wk[w_, k]
            hi = occ_hi_wk[w_, k]
            for j in range(hi - lo):
                ops.append((int(tcb_wk[w_, k] + lo + j), int(ob + j)))
        assert ops, f"window {w_} has no occurrences"
        occs = [o for _, o in ops]
        assert occs == list(range(occs[0], occs[0] + len(occs))), w_
        sched[w_] = ops

    return eidx16, s_all, batches, calls, sched, call_cnt, tc_total, occ_total


def _build_program(batches, calls, sched, call_cnt, tc_total, occ_total):
    import concourse.bacc as bacc
    import concourse.tile as tile
    from concourse import mybir

    f32 = mybir.dt.float32
    bf16 = mybir.dt.bfloat16
    i16 = mybir.dt.int16

    nc = bacc.Bacc("TRN2", target_bir_lowering=False, debug=False,
                   num_swdge_queues=NSWQ)

    xin = nc.declare_dram_parameter("xin", [N_NODES, C], bf16, isOutput=False)
    eidx_d = nc.declare_dram_parameter("eidx", [128, tc_total * 8], i16, isOutput=False)
    s_d = nc.declare_dram_parameter("smat", [128, occ_total * WIN], bf16, isOutput=False)
    wt_d = nc.declare_dram_parameter("wt", [C, C], bf16, isOutput=False)
    bias_d = nc.declare_dram_parameter("bias", [C, 1], f32, isOutput=False)
    out_d = nc.declare_dram_parameter("out", [C, RPC], f32, isOutput=True)

    # S slices are loaded per WINDOW (occ columns are contiguous per window)
    maxw = max(len(ops) for ops in sched)

    with tile.TileContext(nc) as tc:
        with (
            tc.tile_pool(name="consts", bufs=1) as consts,
            tc.tile_pool(name="swp", bufs=4) as swp,
            tc.tile_pool(name="xgp", bufs=3) as xgp,
            tc.tile_pool(name="aggp", bufs=3) as aggp,
            tc.tile_pool(name="op", bufs=3) as op,
            tc.tile_pool(name="ps1", bufs=3, space="PSUM") as ps1,
            tc.tile_pool(name="ps2", bufs=3, space="PSUM") as ps2,
        ):
            wt_t = consts.tile([C, C], bf16)
            bias_t = consts.tile([C, 1], f32)
            nc.sync.dma_start(wt_t[:], wt_d[:])
            nc.sync.dma_start(bias_t[:], bias_d[:])
            # preload ALL gather index planes once: gathers then never wait
            # on a per-batch metadata DMA queued behind the big S loads.
            # Batch 0's slice loads first so the first gather starts early.
            b0_tiles = batches[0][2]
            eidx_all = consts.tile([128, tc_total * 8], i16)
            nc.sync.dma_start(
                eidx_all[:, : b0_tiles * 8], eidx_d[:, : b0_tiles * 8]
            )
            nc.sync.dma_start(
                eidx_all[:, b0_tiles * 8 :], eidx_d[:, b0_tiles * 8 :]
            )

            for bi, (w0, nwin, btiles, bo0, bnocc) in enumerate(batches):
                bcalls = calls[bi]
                c0 = bcalls[0][1]  # first tile col of batch

                # SWDGE descriptor generation is serialized across queues
                # with a ~2.4us fixed cost per call, so issue exactly one
                # call per chunk (4/batch); queues only spread the drain.
                xg = xgp.tile([128, CB, C], bf16, tag="xg")
                for qi, (k, cb0, nt) in enumerate(bcalls):
                    kb = k * CHUNK
                    rows_k = min(CHUNK, N_NODES - kb)
                    lb = cb0 - c0
                    nc.gpsimd.dma_gather(
                        xg[:, lb : lb + nt, :],
                        xin[kb : kb + rows_k, :],
                        eidx_all[:, cb0 * 8 : (cb0 + nt) * 8],
                        nt * 128,
                        nt * 128,
                        C,
                        single_packet=SINGLE_PACKET,
                        queue_num=qi % NSWQ,
                    )

                for w in range(w0, w0 + nwin):
                    ops = sched[w]
                    rw = min(WIN, RPC - w * WIN)
                    ow0 = ops[0][1]  # first (global) occ of this window
                    s_w = swp.tile([128, maxw * WIN], bf16, tag="sw")
                    nc.sync.dma_start(
                        s_w[:, : len(ops) * WIN],
                        s_d[:, ow0 * WIN : (ow0 + len(ops)) * WIN],
                    )
                    psum1 = ps1.tile([C, WIN], f32, tag="psum1")
                    for ti, (xcol, occ) in enumerate(ops):
                        lx = xcol - c0
                        lo_ = occ - ow0
                        nc.tensor.matmul(
                            psum1[:],
                            lhsT=xg[:, lx, :],
                            rhs=s_w[:, lo_ * WIN : (lo_ + 1) * WIN],
                            start=(ti == 0),
                            stop=(ti == len(ops) - 1),
                        )
                    aggT = aggp.tile([C, WIN], bf16, tag="aggT")
                    nc.scalar.copy(aggT[:], psum1[:])
                    psum2 = ps2.tile([C, WIN], f32, tag="psum2")
                    nc.tensor.matmul(
                        psum2[:], lhsT=wt_t[:], rhs=aggT[:], start=True, stop=True
                    )
                    outw = op.tile([C, WIN], f32, tag="outw")
                    nc.scalar.activation(
                        outw[:, :rw],
                        psum2[:, :rw],
                        mybir.ActivationFunctionType.Identity,
                        bias=bias_t[:, 0:1],
                        scale=1.0,
                    )
                    nc.sync.dma_start(
                        out_d[:, w * WIN : w * WIN + rw], outw[:, :rw]
                    )

    nc.compile()
    return nc


def kernel(x, rows, cols, vals, W, b):
    from concourse.bass_utils import run_bass_kernel_spmd
    import ml_dtypes

    x = np.ascontiguousarray(np.asarray(x), dtype=np.float32)
    W = np.asarray(W).astype(np.float32)
    b = np.asarray(b).astype(np.float32)

    (eidx16, s_all, batches, calls, sched, call_cnt,
     tc_total, occ_total) = _host_prep(rows, cols, vals)

    x_dev = x.astype(ml_dtypes.bfloat16)
    wt = np.ascontiguousarray(W.T).astype(ml_dtypes.bfloat16)   # [cin, cout]
    bias = np.ascontiguousarray(b[:, None]).astype(np.float32)  # [cout, 1]

    nc = _build_program(batches, calls, sched, call_cnt, tc_total, occ_total)

    in_maps = [
        {
            "xin": x_dev,
            "eidx": np.ascontiguousarray(eidx16[c]),
            "smat": s_all[c],
            "wt": wt,
            "bias": bias,
        }
        for c in range(N_CORES)
    ]

    res = run_bass_kernel_spmd(nc, in_maps, list(range(N_CORES)), trace=TRACE)
    LAST_RESULT["exec_time_ns"] = res.exec_time_ns
    LAST_RESULT["results"] = res

    out = np.empty((N_NODES, C), dtype=np.float32)
    for c in range(N_CORES):
        out[c * RPC : (c + 1) * RPC] = res.results[c]["out"].T
    return out


# revision 26
# speedup vs baseline: 1.0913x; 1.0913x over previous
"""Trainium2 Bass kernel for nn_MeshConv (COO SpMM + 128x128 Linear).

out[r, :] = (sum_{e: rows[e]==r} vals[e] * x[cols[e], :]) @ W.T + b

Strategy (8 NeuronCores, one SPMD program):
  - Row-shard: core c owns output rows [c*12500, (c+1)*12500); x, W, b
    are replicated per core, so no collectives are needed.
  - Host packs each core's edges densely per (batch of windows, 32768-row
    column chunk) into 128-slot gather tiles, sorted by output window.
    Per-core shortfall vs the SPMD max is padded at the call tail with
    idx=-1 (the gather ucode skips trailing negatives).
  - The selection matrices S[slot, row] = val_e * onehot(lrow_e) are
    fully PRECOMPUTED ON THE HOST (indices and values are kernel inputs,
    so S is just data) and streamed from HBM — no on-device S build.
  - Device, per batch: dma_gather x rows per chunk into SBUF (bf16),
    stream S tiles, and accumulate aggT[cin, row] = sum_t Xg_t^T @ S_t
    per window in PSUM on TensorE.  Then outT_w = W @ agg = wt^T @ aggT
    with one more matmul, and the Scalar engine fuses the per-partition
    bias while copying PSUM -> SBUF.  Output is stored transposed
    [C, rows] and untransposed on the host.
"""

import os
import sys

for _p in ("/opt/trn_rl_repo",):
    if _p not in sys.path:
        sys.path.insert(0, _p)

import numpy as np

# --- problem constants ---
N_NODES = 100000
C = 128
N_CORES = 8
RPC = N_NODES // N_CORES          # rows per core: 12500
WIN = 128                         # output window = PSUM partition dim
NW = (RPC + WIN - 1) // WIN       # 98 windows (last has 84 rows)
CHUNK = 32768                     # column chunk (int16 gather indices)
NK = (N_NODES + CHUNK - 1) // CHUNK  # 4

CB = int(os.environ.get("MESHCONV_CB", "160"))        # gather tile cols per batch
OCCCAP = int(os.environ.get("MESHCONV_OCCCAP", "400"))  # S cols per batch (loose)
NSWQ = int(os.environ.get("MESHCONV_NSWQ", "4"))
SINGLE_PACKET = os.environ.get("MESHCONV_SP", "0") == "1"

TRACE = False          # set by test.py for profiling runs
LAST_RESULT = {}       # test.py reads exec_time_ns etc. from here


def _host_prep(rows, cols, vals):
    """Pack edges densely per (core, batch, chunk); precompute S tiles.

    Returns:
      eidx16  [NC, 128, TC*8] int16   wrapped gather-index planes
      s_all   [NC, 128, OCC*WIN] bf16-able f32  selection matrices
      batches list of (w0, nwin, tile_cols, occ0, nocc)
      calls   list per batch of (k, tile_col_base, ntiles)
      sched   per window: list of (xg_col, occ_idx) matmul operands
      tc_total, occ_total
    """
    rows = np.asarray(rows).astype(np.int64)
    cols = np.asarray(cols).astype(np.int64)
    vals = np.asarray(vals).astype(np.float32)
    E = len(rows)

    core = rows // RPC
    lr = rows - core * RPC
    win = lr // WIN
    lrow = lr - win * WIN
    chunk = cols // CHUNK
    cidx = cols - chunk * CHUNK

    # counts per (core, window, chunk)
    gid = (core * NW + win) * NK + chunk
    cnt = np.bincount(gid, minlength=N_CORES * NW * NK).reshape(N_CORES, NW, NK)

    # --- greedy batching of consecutive windows ---
    # For a candidate batch [w0, w): per chunk k, per core c the edges are
    # packed densely; tiles(k) = ceil(max_c sum_w cnt/128).  Occurrences:
    # for each (w, k) the union tile range over cores.
    def batch_stats(w0, w1):
        tiles = np.zeros(NK, dtype=np.int64)
        nocc = 0
        for k in range(NK):
            tot = cnt[:, w0:w1, k].sum(axis=1)           # [NC]
            tiles[k] = -(-tot.max() // 128)
            cum = np.cumsum(cnt[:, w0:w1, k], axis=1)    # inclusive
            start = cum - cnt[:, w0:w1, k]
            end = cum
            any_e = (cnt[:, w0:w1, k].max(axis=0) > 0)
            lo = np.where(any_e, (start // 128).min(axis=0), 0)
            hi = np.where(any_e, -(-end.max(axis=0) // 128), 0)
            nocc += int(np.maximum(hi - lo, 0).sum())
        return int(tiles.sum()), nocc, tiles

    batches_w = []
    w = 0
    while w < NW:
        w0 = w
        w1 = w0 + 1
        while w1 < NW:
            tc, no, _ = batch_stats(w0, w1 + 1)
            if tc > CB or no > OCCCAP:
                break
            w1 += 1
        tc, no, tiles = batch_stats(w0, w1)
        assert tc <= CB and no <= OCCCAP, (w0, w1, tc, no)
        batches_w.append((w0, w1, tiles))
        w = w1
    # keep the FINAL batch tiny: the pipeline tail (last gather drain +
    # matmuls + stores, which nothing overlaps) scales with its size
    if batches_w and batches_w[-1][1] - batches_w[-1][0] > 2:
        w0, w1, _ = batches_w.pop()
        _, _, ta = batch_stats(w0, w1 - 2)
        _, _, tb = batch_stats(w1 - 2, w1)
        batches_w.append((w0, w1 - 2, ta))
        batches_w.append((w1 - 2, w1, tb))


    # --- global tile-column / occurrence layout ---
    batches = []
    calls = []          # per batch: list of (k, col_base, ntiles)
    sched = [None] * NW  # per window: list of (xg_local_col, occ_local, global refs)
    tilecolbase = np.full((len(batches_w), NK), -1, dtype=np.int64)
    occ_base = {}       # (b, w, k) -> (occ_global_base, lo_tile)
    tc_total = 0
    occ_total = 0
    for bi, (w0, w1, tiles) in enumerate(batches_w):
        bcalls = []
        bt0 = tc_total
        bo0 = occ_total
        for k in range(NK):
            if tiles[k] > 0:
                tilecolbase[bi, k] = tc_total
                bcalls.append((k, tc_total, int(tiles[k])))
                tc_total += int(tiles[k])
        # occurrences ordered by (w, k, t)
        for w_ in range(w0, w1):
            for k in range(NK):
                if tiles[k] == 0:
                    continue
                tot = cnt[:, w0:w1, k]
                cum = np.cumsum(tot, axis=1)
                start = cum[:, w_ - w0] - tot[:, w_ - w0]
                end = cum[:, w_ - w0]
                if tot[:, w_ - w0].max() == 0:
                    continue
                lo = int((start // 128).min())
                hi = int(-(-end.max() // 128))
                occ_base[(bi, w_, k)] = (occ_total, lo)
                occ_total += hi - lo
        batches.append((w0, w1 - w0, tc_total - bt0, bo0, occ_total - bo0))
        calls.append(bcalls)

    # --- slot assignment for every edge ---
    wb = np.zeros(NW, dtype=np.int64)           # window -> batch
    for bi, (w0, w1, _) in enumerate(batches_w):
        wb[w0:w1] = bi
    eb = wb[win]
    # sort edges by (core, batch, chunk, window); rank within call = slot
    order = np.lexsort((win, chunk, eb, core))
    core_s = core[order]
    eb_s = eb[order]
    chunk_s = chunk[order]
    callid = (core_s * len(batches) + eb_s) * NK + chunk_s
    ncalls = N_CORES * len(batches) * NK
    start_of = np.searchsorted(callid, np.arange(ncalls), side="left")
    slot = np.arange(E) - start_of[callid]

    # per-call max real count -> per-core pad at the tail
    call_cnt = np.bincount(callid, minlength=ncalls).reshape(
        N_CORES, len(batches), NK
    )

    tile_s = slot // 128
    p_s = slot - tile_s * 128

    # vectorized lookup tables [NW, NK]
    tcb_wk = np.zeros((NW, NK), dtype=np.int64)       # tile col base
    occ_base_wk = np.full((NW, NK), -1, dtype=np.int64)
    occ_lo_wk = np.zeros((NW, NK), dtype=np.int64)
    occ_hi_wk = np.zeros((NW, NK), dtype=np.int64)
    for bi, (w0, w1, tiles) in enumerate(batches_w):
        for k in range(NK):
            tcb_wk[w0:w1, k] = tilecolbase[bi, k]
    for (bi, w_, k), (ob, lo) in occ_base.items():
        occ_base_wk[w_, k] = ob
        occ_lo_wk[w_, k] = lo
    for bi, (w0, w1, tiles) in enumerate(batches_w):
        for k in range(NK):
            tot = cnt[:, w0:w1, k]
            cum = np.cumsum(tot, axis=1)
            start = cum - tot
            end = cum
            any_e = tot.max(axis=0) > 0
            hi = np.where(any_e, -(-end.max(axis=0) // 128), 0)
            occ_hi_wk[w0:w1, k] = hi

    # gather index planes: [NC, tc_total, 128] then wrapped.
    # Padding uses idx=0 (gathers row kb+0, masked by S=0): trailing -1
    # truncation would desync the decode-side ring bookkeeping (which uses
    # num_idxs_reg) from the ucode's actual descriptor count.
    sidx = np.zeros((N_CORES, tc_total, 128), dtype=np.int16)
    win_s = win[order]
    gcol = tcb_wk[win_s, chunk_s] + tile_s
    sidx[core_s, gcol, p_s] = cidx[order].astype(np.int16)

    eidx16 = np.zeros((N_CORES, 128, tc_total * 8), dtype=np.int16)
    flat = sidx.reshape(N_CORES, tc_total * 128)
    wrapped = flat.reshape(N_CORES, tc_total * 8, 16).transpose(0, 2, 1)
    eidx16[:] = np.tile(wrapped, (1, 8, 1))

    # --- S matrices ---
    occ_e = occ_base_wk[win_s, chunk_s] + tile_s - occ_lo_wk[win_s, chunk_s]
    assert (occ_base_wk[win_s, chunk_s] >= 0).all()
    assert (occ_e >= 0).all() and (occ_e < occ_total).all()
    assert (tile_s >= occ_lo_wk[win_s, chunk_s]).all()
    assert (tile_s < occ_hi_wk[win_s, chunk_s]).all()
    import ml_dtypes

    lrow_s = lrow[order]
    vals_s = vals[order]
    s_all = []
    for c in range(N_CORES):
        m = core_s == c
        sc = np.zeros((occ_total, 128, WIN), dtype=np.float32)
        sc[occ_e[m], p_s[m], lrow_s[m]] = vals_s[m]
        s_all.append(
            np.ascontiguousarray(
                sc.transpose(1, 0, 2).reshape(128, occ_total * WIN)
            ).astype(ml_dtypes.bfloat16)
        )

    # --- matmul schedule per window ---
    for w_ in range(NW):
        ops = []
        for k in range(NK):
            if occ_base_wk[w_, k] < 0:
                continue
            ob = occ_base_wk[w_, k]
            lo = occ_lo_wk[w_, k]
            hi = occ_hi_wk[w_, k]
            for j in range(hi - lo):
                ops.append((int(tcb_wk[w_, k] + lo + j), int(ob + j)))
        assert ops, f"window {w_} has no occurrences"
        occs = [o for _, o in ops]
        assert occs == list(range(occs[0], occs[0] + len(occs))), w_
        sched[w_] = ops

    return eidx16, s_all, batches, calls, sched, call_cnt, tc_total, occ_total


def _build_program(batches, calls, sched, call_cnt, tc_total, occ_total):
    import concourse.bacc as bacc
    import concourse.tile as tile
    from concourse import mybir

    f32 = mybir.dt.float32
    bf16 = mybir.dt.bfloat16
    i16 = mybir.dt.int16

    nc = bacc.Bacc("TRN2", target_bir_lowering=False, debug=False,
                   num_swdge_queues=NSWQ)

    xin = nc.declare_dram_parameter("xin", [N_NODES, C], bf16, isOutput=False)
    eidx_d = nc.declare_dram_parameter("eidx", [128, tc_total * 8], i16, isOutput=False)
    s_d = nc.declare_dram_parameter("smat", [128, occ_total * WIN], bf16, isOutput=False)
    wt_d = nc.declare_dram_parameter("wt", [C, C], bf16, isOutput=False)
    bias_d = nc.declare_dram_parameter("bias", [C, 1], f32, isOutput=False)
    out_d = nc.declare_dram_parameter("out", [C, RPC], f32, isOutput=True)

    # S slices are loaded per WINDOW (occ columns are contiguous per window)
    maxw = max(len(ops) for ops in sched)
    maxnw = max(b[1] for b in batches)

    with tile.TileContext(nc) as tc:
        with (
            tc.tile_pool(name="consts", bufs=1) as consts,
            tc.tile_pool(name="swp", bufs=4) as swp,
            tc.tile_pool(name="xgp", bufs=3) as xgp,
            tc.tile_pool(name="aggp", bufs=3) as aggp,
            tc.tile_pool(name="op", bufs=2) as op,
            tc.tile_pool(name="ps1", bufs=3, space="PSUM") as ps1,
            tc.tile_pool(name="ps2", bufs=3, space="PSUM") as ps2,
        ):
            wt_t = consts.tile([C, C], bf16)
            bias_t = consts.tile([C, 1], f32)
            nc.sync.dma_start(wt_t[:], wt_d[:])
            nc.sync.dma_start(bias_t[:], bias_d[:])
            # preload ALL gather index planes once: gathers then never wait
            # on a per-batch metadata DMA queued behind the big S loads.
            # Batch 0's slice loads first so the first gather starts early.
            b0_tiles = batches[0][2]
            eidx_all = consts.tile([128, tc_total * 8], i16)
            nc.sync.dma_start(
                eidx_all[:, : b0_tiles * 8], eidx_d[:, : b0_tiles * 8]
            )
            nc.sync.dma_start(
                eidx_all[:, b0_tiles * 8 :], eidx_d[:, b0_tiles * 8 :]
            )

            for bi, (w0, nwin, btiles, bo0, bnocc) in enumerate(batches):
                bcalls = calls[bi]
                c0 = bcalls[0][1]  # first tile col of batch

                # SWDGE descriptor generation is serialized across queues
                # with a ~2.4us fixed cost per call, so issue exactly one
                # call per chunk (4/batch); queues only spread the drain.
                xg = xgp.tile([128, CB, C], bf16, tag="xg")
                for qi, (k, cb0, nt) in enumerate(bcalls):
                    kb = k * CHUNK
                    rows_k = min(CHUNK, N_NODES - kb)
                    lb = cb0 - c0
                    nc.gpsimd.dma_gather(
                        xg[:, lb : lb + nt, :],
                        xin[kb : kb + rows_k, :],
                        eidx_all[:, cb0 * 8 : (cb0 + nt) * 8],
                        nt * 128,
                        nt * 128,
                        C,
                        single_packet=SINGLE_PACKET,
                        queue_num=qi % NSWQ,
                    )

                # one output tile + one store per batch (fewer DMAs/sems)
                bw = min(nwin * WIN, RPC - w0 * WIN)
                outw = op.tile([C, maxnw * WIN], f32, tag="outw")
                for w in range(w0, w0 + nwin):
                    ops = sched[w]
                    rw = min(WIN, RPC - w * WIN)
                    ow0 = ops[0][1]  # first (global) occ of this window
                    s_w = swp.tile([128, maxw * WIN], bf16, tag="sw")
                    nc.scalar.dma_start(
                        s_w[:, : len(ops) * WIN],
                        s_d[:, ow0 * WIN : (ow0 + len(ops)) * WIN],
                    )
                    psum1 = ps1.tile([C, WIN], f32, tag="psum1")
                    for ti, (xcol, occ) in enumerate(ops):
                        lx = xcol - c0
                        lo_ = occ - ow0
                        nc.tensor.matmul(
                            psum1[:],
                            lhsT=xg[:, lx, :],
                            rhs=s_w[:, lo_ * WIN : (lo_ + 1) * WIN],
                            start=(ti == 0),
                            stop=(ti == len(ops) - 1),
                        )
                    aggT = aggp.tile([C, WIN], bf16, tag="aggT")
                    nc.scalar.copy(aggT[:], psum1[:])
                    psum2 = ps2.tile([C, WIN], f32, tag="psum2")
                    nc.tensor.matmul(
                        psum2[:], lhsT=wt_t[:], rhs=aggT[:], start=True, stop=True
                    )
                    lw = (w - w0) * WIN
                    nc.scalar.activation(
                        outw[:, lw : lw + rw],
                        psum2[:, :rw],
                        mybir.ActivationFunctionType.Identity,
                        bias=bias_t[:, 0:1],
                        scale=1.0,
                    )
                nc.sync.dma_start(
                    out_d[:, w0 * WIN : w0 * WIN + bw], outw[:, :bw]
                )

    nc.compile()
    return nc


def kernel(x, rows, cols, vals, W, b):
    from concourse.bass_utils import run_bass_kernel_spmd
    import ml_dtypes

    x = np.ascontiguousarray(np.asarray(x), dtype=np.float32)
    W = np.asarray(W).astype(np.float32)
    b = np.asarray(b).astype(np.float32)

    (eidx16, s_all, batches, calls, sched, call_cnt,
     tc_total, occ_total) = _host_prep(rows, cols, vals)

    x_dev = x.astype(ml_dtypes.bfloat16)
    wt = np.ascontiguousarray(W.T).astype(ml_dtypes.bfloat16)   # [cin, cout]
    bias = np.ascontiguousarray(b[:, None]).astype(np.float32)  # [cout, 1]

    nc = _build_program(batches, calls, sched, call_cnt, tc_total, occ_total)

    in_maps = [
        {
            "xin": x_dev,
            "eidx": np.ascontiguousarray(eidx16[c]),
            "smat": s_all[c],
            "wt": wt,
            "bias": bias,
        }
        for c in range(N_CORES)
    ]

    res = run_bass_kernel_spmd(nc, in_maps, list(range(N_CORES)), trace=TRACE)
    LAST_RESULT["exec_time_ns"] = res.exec_time_ns
    LAST_RESULT["results"] = res

    out = np.empty((N_NODES, C), dtype=np.float32)
    for c in range(N_CORES):
        out[c * RPC : (c + 1) * RPC] = res.results[c]["out"].T
    return out
